# revision 8
# baseline (speedup 1.0000x reference)
"""Trainium2 Bass kernel for CausalWanSelfAttention (block-causal window attention).

Geometry: B=1, S=6240, DIM=1536, H=12 heads x D=128, frames of L=1560 tokens,
window = current + previous frame.

Sharding over 8 NeuronCores (sequence-parallel with KV AllGather):
  - core c owns tokens [780c, 780c+780): computes fused QKV for them
    (weights replicated), full-dim RMSNorm + RoPE locally,
  - AllGathers normed/roped K (feature-major [1536,780]) and V
    (token-major [780,1536]) across cores,
  - attends its 780 queries to its 2-frame KV window (3120 tokens) read from
    the gathered buffers at per-core dynamic offsets. Frame-0 cores use a
    duplicated-frame window (softmax over a duplicated key set equals softmax
    over the single set exactly), so no masking is needed anywhere,
  - local output projection (all heads of a token live on one core).

Layouts: q,k are feature-major [d, token] (RMSNorm partition reductions and
per-token broadcasts are done with small PE matmuls); v is token-major
[token, d] so it can be the stationary operand of the PV matmul directly.
The head-dim order of q,k is de-interleaved on the host (even rotary lanes
first, odd lanes second) so RoPE works on contiguous partition halves; the
q.k dot product is invariant to this permutation.
"""

import numpy as np

import concourse.bass as bass
import concourse.bacc as bacc
import concourse.mybir as mybir
import concourse.tile as tile
from concourse import bass_utils

F32 = mybir.dt.float32
U32 = mybir.dt.uint32
AF = mybir.ActivationFunctionType
ALU = mybir.AluOpType

# Geometry (hardcoded per the problem spec).
S, DIM, H, D = 6240, 1536, 12, 128
HD = H * D                      # 1536
L = 1560                        # frame length
NCORES = 8
T = S // NCORES                 # 780 tokens per core
QG = 390                        # query/token group: 2 per core, fits one PSUM bank
EPS = 1e-6
KQ = DIM // 128                 # 12 contraction chunks for the QKV matmuls
# token sub-tiles within a 780-token rank block: 6x128 + 1x12
TOK_SPLITS = [(i * 128, min(128, T - i * 128)) for i in range((T + 127) // 128)]


def _build_nc():
    nc = bacc.Bacc("TRN2", target_bir_lowering=False, debug=False,
                   enable_asserts=True, num_devices=NCORES)

    # ---- per-core inputs ----
    hidT = nc.dram_tensor("hidT", [DIM + 1, T], F32, kind="ExternalInput").ap()
    csd = nc.dram_tensor("csd", [128, 2 * T], F32, kind="ExternalInput").ap()
    wink = nc.dram_tensor("wink", [1, 4], U32, kind="ExternalInput").ap()  # 1536*w
    winv = nc.dram_tensor("winv", [1, 4], U32, kind="ExternalInput").ap()  # 780*w

    # ---- replicated inputs ----
    WqkT = nc.dram_tensor("WqkT", [DIM, 2 * HD], F32, kind="ExternalInput").ap()
    WvTa = nc.dram_tensor("WvTa", [DIM + 1, HD], F32, kind="ExternalInput").ap()
    bqk = nc.dram_tensor("bqk", [128, 2 * H], F32, kind="ExternalInput").ap()
    grow = nc.dram_tensor("grow", [1, 2 * HD], F32, kind="ExternalInput").ap()
    WoT = nc.dram_tensor("WoT", [HD, DIM], F32, kind="ExternalInput").ap()
    bo = nc.dram_tensor("bo", [128, DIM // 128], F32, kind="ExternalInput").ap()

    # ---- output (feature-major; host transposes back) ----
    outT = nc.dram_tensor("outT", [DIM, T], F32, kind="ExternalOutput").ap()

    # ---- internal DRAM for the collectives ----
    kcon = nc.dram_tensor("kcon", [HD, T], F32)
    vcon = nc.dram_tensor("vcon", [T, HD], F32)
    gk = nc.dram_tensor("gk", [NCORES * HD, T], F32, addr_space="Shared")
    gv = nc.dram_tensor("gv", [NCORES * T, HD], F32, addr_space="Shared")

    with tile.TileContext(nc) as tc:
        _emit(nc, tc, hidT, csd, wink, winv, WqkT, WvTa, bqk, grow,
              WoT, bo, outT, kcon, vcon, gk, gv)
    nc.compile()
    return nc


def _emit(nc, tc, hidT, csd, wink, winv, WqkT, WvTa, bqk, grow,
          WoT, bo, outT, kcon, vcon, gk, gv):
    # window base registers (element offsets into gk / gv axis 0)
    kregs, vregs = [], []
    for i in range(4):
        rk = nc.alloc_registers(f"wk{i}")
        nc.regs_load(rk, wink.tensor[0:1, i:i + 1])
        kregs.append(nc.snap(rk, donate=True, min_val=0,
                             max_val=(NCORES - 1) * HD))
        rv = nc.alloc_registers(f"wv{i}")
        nc.regs_load(rv, winv.tensor[0:1, i:i + 1])
        vregs.append(nc.snap(rv, donate=True, min_val=0,
                             max_val=(NCORES - 1) * T))

    with (
        tc.tile_pool(name="const", bufs=1) as const,
        tc.tile_pool(name="qsb", bufs=1) as q_pool,       # roped q, per head
        tc.tile_pool(name="attsb", bufs=1) as att_pool,   # k (early) + attn out
    ):
        ones_col = const.tile([128, 1], F32)          # denominator lhsT
        nc.vector.memset(ones_col, 1.0)
        ones_row = const.tile([1, 128], F32)          # partition-broadcast lhsT
        nc.vector.memset(ones_row, 1.0)
        bqk_sb = const.tile([128, 2 * H], F32)
        nc.sync.dma_start(bqk_sb, bqk)
        bo_sb = const.tile([128, DIM // 128], F32)
        nc.sync.dma_start(bo_sb, bo)
        eps_q = const.tile([1, 1], F32)
        nc.vector.memset(eps_q, D * EPS)
        eps_k = const.tile([1, 1], F32)
        nc.vector.memset(eps_k, EPS)

        # ================= phase A: QKV projections, norms, rope, gathers ====
        with (
            tc.tile_pool(name="hid", bufs=1) as hid_pool,
            tc.tile_pool(name="wls", bufs=6) as wl_pool,
            tc.tile_pool(name="vws", bufs=1) as vw_pool,
            tc.tile_pool(name="tmp", bufs=1) as tmp_pool,
            tc.tile_pool(name="ropet", bufs=1) as rope_pool,
            tc.tile_pool(name="small", bufs=1) as small_pool,
            tc.tile_pool(name="csp", bufs=1) as cs_pool,
            tc.tile_pool(name="qkps", bufs=4, space="PSUM") as ps_pool,
            tc.tile_pool(name="scps", bufs=2, space="PSUM") as sc_ps_pool,
            tc.tile_pool(name="redps", bufs=2, space="PSUM") as red_ps_pool,
        ):
            # [cos;cos] in cols 0:T, [sin;-sin] in cols T:2T
            cs_sb = cs_pool.tile([128, 2 * T], F32)
            nc.sync.dma_start(cs_sb, csd)

            hid = [hid_pool.tile([128, T], F32, tag=f"hid{i}", name=f"hid{i}")
                   for i in range(KQ)]
            for i in range(KQ):
                nc.sync.dma_start(hid[i], hidT.tensor[128 * i:128 * (i + 1), :])
            hid_ones = hid_pool.tile([1, T], F32, tag="hid_ones")
            nc.sync.dma_start(hid_ones, hidT.tensor[DIM:DIM + 1, :])

            def qk_path(which, dest_tiles):
                mlo = H if which == "k" else 0
                g_row = small_pool.tile([1, HD], F32, tag="grow")
                nc.sync.dma_start(g_row, grow.tensor[0:1, mlo * 128:
                                                     (mlo + H) * 128])
                # --- projection + biased evac + sum of squares ---
                ssq = small_pool.tile([128, T], F32, tag="ssq")
                for mi in range(H):
                    m = mlo + mi
                    dest = dest_tiles[mi]
                    tsq = tmp_pool.tile([128, T], F32, tag="tsq")
                    for g in range(2):
                        qs = slice(g * QG, (g + 1) * QG)
                        ps = ps_pool.tile([128, QG], F32, tag="qkps")
                        for kc in range(KQ):
                            w_sb = wl_pool.tile([128, 128], F32, tag="wqk")
                            nc.sync.dma_start(
                                w_sb, WqkT.tensor[128 * kc:128 * (kc + 1),
                                                  128 * m:128 * (m + 1)])
                            nc.tensor.matmul(ps, w_sb, hid[kc][:, qs],
                                             start=(kc == 0),
                                             stop=(kc == KQ - 1))
                        nc.scalar.activation(dest[:, qs], ps, AF.Identity,
                                             bias=bqk_sb[:, m:m + 1])
                        nc.scalar.activation(tsq[:, qs], ps, AF.Square,
                                             bias=bqk_sb[:, m:m + 1])
                    if mi == 0:
                        nc.vector.tensor_copy(ssq, tsq)
                    else:
                        nc.vector.tensor_tensor(ssq, ssq, tsq, ALU.add)
                # --- rms scale: s = 1/sqrt(mean+eps)  (x 1/sqrt(D) for q) ---
                sq_scale = (D / DIM) if which == "q" else (1.0 / DIM)
                sq_bias = eps_q if which == "q" else eps_k
                inv = small_pool.tile([1, T], F32, tag="inv")
                for g in range(2):
                    qs = slice(g * QG, (g + 1) * QG)
                    red = red_ps_pool.tile([1, QG], F32, tag="redps")
                    nc.tensor.matmul(red, ones_col, ssq[:, qs], start=True,
                                     stop=True)
                    rt = small_pool.tile([1, QG], F32, tag="rt")
                    nc.scalar.activation(rt, red, AF.Sqrt, bias=sq_bias,
                                         scale=sq_scale)
                    nc.vector.reciprocal(inv[:, qs], rt)
                # --- scale (in place) + rope (in place) per head chunk ---
                for mi in range(H):
                    dest = dest_tiles[mi]
                    for g in range(2):
                        qs = slice(g * QG, (g + 1) * QG)
                        scp = sc_ps_pool.tile([128, QG], F32, tag="scps")
                        nc.tensor.matmul(scp, g_row[:, 128 * mi:128 * (mi + 1)],
                                         inv[:, qs], start=True, stop=True)
                        nc.vector.tensor_tensor(dest[:, qs], dest[:, qs], scp,
                                                ALU.mult)
                        cc = cs_sb[:, g * QG:(g + 1) * QG]
                        ssg = cs_sb[:, T + g * QG:T + (g + 1) * QG]
                        ta = rope_pool.tile([128, QG], F32, tag="ra")
                        tb = rope_pool.tile([128, QG], F32, tag="rb")
                        sw = rope_pool.tile([128, QG], F32, tag="rsw")
                        nc.vector.tensor_tensor(ta, dest[:, qs], cc, ALU.mult)
                        nc.vector.tensor_tensor(tb, dest[:, qs], ssg, ALU.mult)
                        nc.sync.dma_start(sw[0:64, :], tb[64:128, :])
                        nc.sync.dma_start(sw[64:128, :], tb[0:64, :])
                        nc.vector.tensor_tensor(dest[:, qs], ta, sw, ALU.add)

            # ---- k first (feeds the first collective) ----
            k_tiles = [att_pool.tile([128, T], F32, tag=f"att{h}", name=f"kt{h}")
                       for h in range(H)]
            qk_path("k", k_tiles)
            for mi in range(H):
                nc.sync.dma_start(kcon.ap()[128 * mi:128 * (mi + 1), :],
                                  k_tiles[mi])
            nc.gpsimd.collective_compute(
                "AllGather", ALU.bypass, replica_groups=[list(range(NCORES))],
                ins=[kcon.ap()], outs=[gk.ap()])

            # ---- v: token-major, contraction over dim chunks + bias row ----
            for og in range(HD // 512):
                vb = small_pool.tile([1, 512], F32, tag="vb")
                nc.sync.dma_start(
                    vb, WvTa.tensor[DIM:DIM + 1, 512 * og:512 * (og + 1)])
                vw = [vw_pool.tile([128, 512], F32, tag=f"vw{kc}", name=f"vw{kc}")
                      for kc in range(KQ)]
                for kc in range(KQ):
                    nc.sync.dma_start(
                        vw[kc], WvTa.tensor[128 * kc:128 * (kc + 1),
                                            512 * og:512 * (og + 1)])
                for (t0, tn_) in TOK_SPLITS:
                    ps = ps_pool.tile([128, 512], F32, tag="qkps")
                    for kc in range(KQ):
                        nc.tensor.matmul(ps[0:tn_, :], hid[kc][:, t0:t0 + tn_],
                                         vw[kc], start=(kc == 0), stop=False)
                    nc.tensor.matmul(ps[0:tn_, :], hid_ones[:, t0:t0 + tn_], vb,
                                     start=False, stop=True)
                    vsb = tmp_pool.tile([128, 512], F32, tag="vsb")
                    nc.scalar.activation(vsb[0:tn_, :], ps[0:tn_, :], AF.Identity)
                    nc.sync.dma_start(
                        vcon.ap()[t0:t0 + tn_, 512 * og:512 * (og + 1)],
                        vsb[0:tn_, :])
            nc.gpsimd.collective_compute(
                "AllGather", ALU.bypass, replica_groups=[list(range(NCORES))],
                ins=[vcon.ap()], outs=[gv.ap()])

            # ---- q ----
            q_tiles = [q_pool.tile([128, T], F32, tag=f"q{h}", name=f"qt{h}")
                       for h in range(H)]
            qk_path("q", q_tiles)

        # ================= phase B: attention ================================
        with (
            tc.tile_pool(name="kwin", bufs=2) as kv_pool,
            tc.tile_pool(name="vwin", bufs=30) as vt_pool,
            tc.tile_pool(name="probs", bufs=6) as probs_pool,
            tc.tile_pool(name="attm", bufs=2) as attm_pool,
            tc.tile_pool(name="attsc", bufs=3, space="PSUM") as sc_ps,
            tc.tile_pool(name="attout", bufs=2, space="PSUM") as out_ps,
            tc.tile_pool(name="attden", bufs=2, space="PSUM") as den_ps,
            tc.tile_pool(name="attbc", bufs=1, space="PSUM") as bc_ps,
        ):
            att_tiles = []
            for h in range(H):
                ksb = kv_pool.tile([128, 4 * T], F32, tag="ksb")
                for w in range(4):
                    nc.sync.dma_start(
                        ksb[:, w * T:(w + 1) * T],
                        gk[bass.ds(kregs[w] + 128 * h, 128), :])
                vts = []
                for w in range(4):
                    for (t0, tn_) in TOK_SPLITS:
                        vt = vt_pool.tile([128, 128], F32, tag="vt")
                        nc.sync.dma_start(
                            vt[0:tn_, :],
                            gv[bass.ds(vregs[w] + t0, tn_),
                               128 * h:128 * (h + 1)])
                        vts.append((w, t0, tn_, vt))
                ath = att_pool.tile([128, T], F32, tag=f"att{h}")
                att_tiles.append(ath)
                for g in range(2):
                    qs = slice(g * QG, (g + 1) * QG)
                    op = out_ps.tile([128, QG], F32, tag="op")
                    dp = den_ps.tile([1, QG], F32, tag="dp")
                    n_kc = len(vts)
                    for ci, (w, t0, tn_, vt) in enumerate(vts):
                        sp = sc_ps.tile([128, QG], F32, tag="sp")
                        nc.tensor.matmul(
                            sp[0:tn_, :], ksb[:, w * T + t0:w * T + t0 + tn_],
                            q_tiles[h][:, qs], start=True, stop=True)
                        pr = probs_pool.tile([128, QG], F32, tag="pr")
                        nc.scalar.activation(pr[0:tn_, :], sp[0:tn_, :], AF.Exp)
                        nc.tensor.matmul(dp, ones_col[0:tn_, :], pr[0:tn_, :],
                                         start=(ci == 0), stop=(ci == n_kc - 1))
                        nc.tensor.matmul(op, vt[0:tn_, :], pr[0:tn_, :],
                                         start=(ci == 0), stop=(ci == n_kc - 1))
                    dsb = attm_pool.tile([1, QG], F32, tag="dsb")
                    nc.vector.reciprocal(dsb, dp)
                    bc = bc_ps.tile([128, QG], F32, tag="bc")
                    nc.tensor.matmul(bc, ones_row, dsb, start=True, stop=True)
                    osb = attm_pool.tile([128, QG], F32, tag="osb")
                    nc.scalar.activation(osb, op, AF.Identity)
                    nc.vector.tensor_tensor(ath[:, qs], osb, bc, ALU.mult)

        # ================= phase C: output projection ========================
        with (
            tc.tile_pool(name="wos", bufs=3) as wo_pool,
            tc.tile_pool(name="osbp", bufs=2) as o_pool,
            tc.tile_pool(name="opps", bufs=2, space="PSUM") as op_ps,
        ):
            for od in range(DIM // 128):
                wo = wo_pool.tile([128, HD], F32, tag="wo")
                nc.sync.dma_start(
                    wo.rearrange("p (c m) -> p c m", c=H),
                    WoT.tensor[:, 128 * od:128 * (od + 1)].rearrange(
                        "(c p) m -> p c m", p=128))
                ot = o_pool.tile([128, T], F32, tag="ot")
                for g in range(2):
                    qs = slice(g * QG, (g + 1) * QG)
                    ps = op_ps.tile([128, QG], F32, tag="opps")
                    for hc in range(H):
                        nc.tensor.matmul(ps, wo[:, 128 * hc:128 * (hc + 1)],
                                         att_tiles[hc][:, qs],
                                         start=(hc == 0), stop=(hc == H - 1))
                    nc.scalar.activation(ot[:, qs], ps, AF.Identity,
                                         bias=bo_sb[:, od:od + 1])
                nc.sync.dma_start(outT.tensor[128 * od:128 * (od + 1), :], ot)


_CACHED_NC = None


def _get_nc():
    global _CACHED_NC
    if _CACHED_NC is None:
        _CACHED_NC = _build_nc()
    return _CACHED_NC


def _deinterleave(n):
    """Permutation putting even rotary lanes first within each 128-dim head."""
    idx = np.arange(n).reshape(-1, D)
    return np.concatenate([idx[:, 0::2], idx[:, 1::2]], axis=1).reshape(-1)


def kernel(hidden_states, freqs_cos, freqs_sin, W_qkv, b_qkv, gq, gk, W_out,
           b_out):
    hidden_states = np.asarray(hidden_states, dtype=np.float32)
    freqs_cos = np.asarray(freqs_cos, dtype=np.float32)
    freqs_sin = np.asarray(freqs_sin, dtype=np.float32)
    W_qkv = np.asarray(W_qkv, dtype=np.float32)
    b_qkv = np.asarray(b_qkv, dtype=np.float32)
    gq = np.asarray(gq, dtype=np.float32)
    gk = np.asarray(gk, dtype=np.float32)
    W_out = np.asarray(W_out, dtype=np.float32)
    b_out = np.asarray(b_out, dtype=np.float32)

    nc = _get_nc()

    perm = _deinterleave(HD)
    Wq, Wk, Wv = W_qkv[:HD][perm], W_qkv[HD:2 * HD][perm], W_qkv[2 * HD:]
    bq, bk, bv = b_qkv[:HD][perm], b_qkv[HD:2 * HD][perm], b_qkv[2 * HD:]
    gqp, gkp = gq[perm], gk[perm]

    WqkT = np.ascontiguousarray(np.concatenate([Wq, Wk], axis=0).T)  # [1536,3072]
    WvTa = np.concatenate([Wv.T, bv[None, :]], axis=0)               # [1537,1536]
    bqk = np.ascontiguousarray(
        np.concatenate([bq, bk]).reshape(2 * H, 128).T)              # [128, 24]
    grow = np.concatenate([gqp, gkp])[None, :]                       # [1, 3072]
    WoT = np.ascontiguousarray(W_out.T)                              # [1536, 1536]
    bo = np.ascontiguousarray(b_out.reshape(DIM // 128, 128).T)      # [128, 12]

    in_maps = []
    for c in range(NCORES):
        sl = slice(c * T, (c + 1) * T)
        hidT = np.concatenate([
            np.ascontiguousarray(hidden_states[0, sl, :].T),
            np.ones((1, T), np.float32)], axis=0)                    # [1537, 780]
        f = (c * T) // L
        if f == 0:
            win = [0, 1, 0, 1]
        else:
            base = 2 * (f - 1)
            win = [base, base + 1, base + 2, base + 3]
        cc = np.ascontiguousarray(freqs_cos[sl].T)       # [64, 780]
        ss = np.ascontiguousarray(freqs_sin[sl].T)
        csd = np.concatenate([
            np.concatenate([cc, cc], axis=0),
            np.concatenate([ss, -ss], axis=0)], axis=1)      # [128, 1560]
        in_maps.append({
            "hidT": hidT,
            "csd": csd,
            "wink": np.asarray([[w * HD for w in win]], np.uint32),
            "winv": np.asarray([[w * T for w in win]], np.uint32),
            "WqkT": WqkT, "WvTa": WvTa, "bqk": bqk, "grow": grow,
            "WoT": WoT, "bo": bo,
        })

    global _LAST_IN_MAPS
    _LAST_IN_MAPS = in_maps
    res = bass_utils.run_bass_kernel_spmd(nc, in_maps,
                                          core_ids=list(range(NCORES)))
    out = np.empty((1, S, DIM), np.float32)
    for c in range(NCORES):
        out[0, c * T:(c + 1) * T, :] = res.results[c]["outT"].T
    return out


# revision 12
# speedup vs baseline: 3.1369x; 3.1369x over previous
"""Trainium2 Bass kernel for CausalWanSelfAttention (block-causal window attention).

Geometry: B=1, S=6240, DIM=1536, H=12 heads x D=128, frames of L=1560 tokens,
window = current + previous frame.

Sharding over 8 NeuronCores (sequence-parallel with KV AllGather):
  - core c owns tokens [780c, 780c+780): computes fused QKV for them
    (weights replicated), full-dim RMSNorm + RoPE locally,
  - AllGathers normed/roped K (feature-major [1536,780]) and V
    (token-major [780,1536]) across cores in bf16,
  - attends its 780 queries to its 2-frame KV window (3120 tokens) read from
    the gathered buffers at per-core dynamic offsets. Frame-0 cores use a
    duplicated-frame window (softmax over a duplicated key set equals softmax
    over the single set exactly), so no masking is needed anywhere,
  - local output projection (all heads of a token live on one core).

Layouts: q,k are feature-major [d, token] (RMSNorm partition reductions and
per-token broadcasts are done with small PE matmuls); v is token-major
[token, d] so it can be the stationary operand of the PV matmul directly.
The head-dim order of q,k is de-interleaved on the host (even rotary lanes
first, odd lanes second) so RoPE works on contiguous partition halves; the
q.k dot product is invariant to this permutation.

Precision: matmul operands are bf16 (fp32 PSUM accumulation); RMSNorm
statistics, RoPE, softmax normalization run in fp32.
"""

import ml_dtypes
import numpy as np

import concourse.bass as bass
import concourse.bacc as bacc
import concourse.mybir as mybir
import concourse.tile as tile
from concourse import bass_utils

F32 = mybir.dt.float32
BF16 = mybir.dt.bfloat16
U32 = mybir.dt.uint32
AF = mybir.ActivationFunctionType
ALU = mybir.AluOpType
NP_BF16 = ml_dtypes.bfloat16

# Geometry (hardcoded per the problem spec).
S, DIM, H, D = 6240, 1536, 12, 128
HD = H * D                      # 1536
L = 1560                        # frame length
NCORES = 8
T = S // NCORES                 # 780 tokens per core
QG = 390                        # query/token group: 2 per core, fits one PSUM bank
EPS = 1e-6
KQ = DIM // 128                 # 12 contraction chunks for the QKV matmuls
# token sub-tiles within a 780-token rank block: 6x128 + 1x12
TOK_SPLITS = [(i * 128, min(128, T - i * 128)) for i in range((T + 127) // 128)]


def _build_nc():
    nc = bacc.Bacc("TRN2", target_bir_lowering=False, debug=False,
                   enable_asserts=True, num_devices=NCORES)

    # ---- per-core inputs ----
    hidT = nc.dram_tensor("hidT", [DIM + 1, T], BF16, kind="ExternalInput").ap()
    csd = nc.dram_tensor("csd", [128, 2 * T], F32, kind="ExternalInput").ap()
    wink = nc.dram_tensor("wink", [1, 4], U32, kind="ExternalInput").ap()  # 1536*w
    winv = nc.dram_tensor("winv", [1, 4], U32, kind="ExternalInput").ap()  # 780*w

    # ---- replicated inputs ----
    WqkT = nc.dram_tensor("WqkT", [DIM, 2 * HD], BF16, kind="ExternalInput").ap()
    WvTa = nc.dram_tensor("WvTa", [DIM + 1, HD], BF16, kind="ExternalInput").ap()
    bqk = nc.dram_tensor("bqk", [128, 2 * H], F32, kind="ExternalInput").ap()
    grow = nc.dram_tensor("grow", [1, 2 * HD], F32, kind="ExternalInput").ap()
    WoT = nc.dram_tensor("WoT", [HD, DIM], BF16, kind="ExternalInput").ap()
    bo = nc.dram_tensor("bo", [128, DIM // 128], F32, kind="ExternalInput").ap()

    # ---- output (feature-major; host transposes back) ----
    outT = nc.dram_tensor("outT", [DIM, T], F32, kind="ExternalOutput").ap()

    # ---- internal DRAM for the collectives ----
    kcon = nc.dram_tensor("kcon", [HD, T], BF16)
    vcon = nc.dram_tensor("vcon", [T, HD], BF16)
    gk = nc.dram_tensor("gk", [NCORES * HD, T], BF16, addr_space="Shared")
    gv = nc.dram_tensor("gv", [NCORES * T, HD], BF16, addr_space="Shared")

    with tile.TileContext(nc) as tc:
        _emit(nc, tc, hidT, csd, wink, winv, WqkT, WvTa, bqk, grow,
              WoT, bo, outT, kcon, vcon, gk, gv)
    nc.compile()
    return nc


def _emit(nc, tc, hidT, csd, wink, winv, WqkT, WvTa, bqk, grow,
          WoT, bo, outT, kcon, vcon, gk, gv):
    # window base registers (element offsets into gk / gv axis 0)
    kregs, vregs = [], []
    for i in range(4):
        rk = nc.alloc_registers(f"wk{i}")
        nc.regs_load(rk, wink.tensor[0:1, i:i + 1])
        kregs.append(nc.snap(rk, donate=True, min_val=0,
                             max_val=(NCORES - 1) * HD))
        rv = nc.alloc_registers(f"wv{i}")
        nc.regs_load(rv, winv.tensor[0:1, i:i + 1])
        vregs.append(nc.snap(rv, donate=True, min_val=0,
                             max_val=(NCORES - 1) * T))

    GS = (slice(0, QG), slice(QG, 2 * QG))        # token groups in SBUF
    PS2 = (slice(0, QG), slice(512, 512 + QG))    # the two bank-aligned halves

    def act2(out_sb, ps2, func, bias=0.0):
        """One ACT op over both 390-wide halves of a 2-bank PSUM tile."""
        nc.scalar.activation(
            out_sb.rearrange("p (a b) -> p a b", a=2),
            ps2.rearrange("p (a b) -> p a b", a=2)[:, :, 0:QG],
            func, bias=bias)

    with (
        tc.tile_pool(name="const", bufs=1) as const,
        tc.tile_pool(name="qsb", bufs=1) as q_pool,       # roped q (bf16)
        tc.tile_pool(name="attsb", bufs=1) as att_pool,   # k (early) + attn out
    ):
        ones_col = const.tile([128, 1], F32)          # fp32 ones (norm reduce)
        nc.vector.memset(ones_col, 1.0)
        ones_bf = const.tile([128, 1], BF16)          # bf16 ones (denominator)
        nc.vector.memset(ones_bf, 1.0)
        ones_row = const.tile([1, 128], F32)          # partition-broadcast lhsT
        nc.vector.memset(ones_row, 1.0)
        bqk_sb = const.tile([128, 2 * H], F32)
        nc.sync.dma_start(bqk_sb, bqk)
        bo_sb = const.tile([128, DIM // 128], F32)
        nc.sync.dma_start(bo_sb, bo)
        eps_q = const.tile([1, 1], F32)
        nc.vector.memset(eps_q, D * EPS)
        eps_k = const.tile([1, 1], F32)
        nc.vector.memset(eps_k, EPS)

        # ================= phase A: QKV projections, norms, rope, gathers ====
        with (
            tc.tile_pool(name="hid", bufs=1) as hid_pool,
            tc.tile_pool(name="wls", bufs=6) as wl_pool,
            tc.tile_pool(name="vws", bufs=1) as vw_pool,
            tc.tile_pool(name="wrk", bufs=1) as wrk_pool,
            tc.tile_pool(name="tmp", bufs=2) as tmp_pool,
            tc.tile_pool(name="ropet", bufs=2) as rope_pool,
            tc.tile_pool(name="small", bufs=1) as small_pool,
            tc.tile_pool(name="csp", bufs=1) as cs_pool,
            tc.tile_pool(name="qkps", bufs=3, space="PSUM") as ps_pool,
            tc.tile_pool(name="scps", bufs=1, space="PSUM") as sc_ps_pool,
            tc.tile_pool(name="redps", bufs=1, space="PSUM") as red_ps_pool,
        ):
            # [cos;cos] in cols 0:T, [sin;-sin] in cols T:2T
            cs_sb = cs_pool.tile([128, 2 * T], F32)
            nc.sync.dma_start(cs_sb, csd)

            hid = [hid_pool.tile([128, T], BF16, tag=f"hid{i}", name=f"hid{i}")
                   for i in range(KQ)]
            for i in range(KQ):
                nc.sync.dma_start(hid[i], hidT.tensor[128 * i:128 * (i + 1), :])
            hid_ones = hid_pool.tile([1, T], BF16, tag="hid_ones")
            nc.sync.dma_start(hid_ones, hidT.tensor[DIM:DIM + 1, :])

            def qk_path(which, dest_tiles):
                mlo = H if which == "k" else 0
                g_row = small_pool.tile([1, HD], F32, tag="grow")
                nc.sync.dma_start(g_row, grow.tensor[0:1, mlo * 128:
                                                     (mlo + H) * 128])
                # --- projection + biased evac + sum of squares ---
                ssq = small_pool.tile([128, T], F32, tag="ssq")
                works = []
                for mi in range(H):
                    m = mlo + mi
                    work = wrk_pool.tile([128, T], F32, tag=f"work{mi}",
                                          name=f"work{mi}")
                    works.append(work)
                    tsq = tmp_pool.tile([128, T], F32, tag="tsq")
                    ps2 = ps_pool.tile([128, 1024], F32, tag="qkps")
                    for kc in range(KQ):
                        w_sb = wl_pool.tile([128, 128], BF16, tag="wqk")
                        nc.sync.dma_start(
                            w_sb, WqkT.tensor[128 * kc:128 * (kc + 1),
                                              128 * m:128 * (m + 1)])
                        for g in range(2):
                            nc.tensor.matmul(ps2[:, PS2[g]], w_sb,
                                             hid[kc][:, GS[g]],
                                             start=(kc == 0),
                                             stop=(kc == KQ - 1))
                    b = bqk_sb[:, m:m + 1]
                    act2(work, ps2, AF.Identity, bias=b)
                    act2(tsq, ps2, AF.Square, bias=b)
                    if mi == 0:
                        nc.vector.tensor_copy(ssq, tsq)
                    else:
                        nc.vector.tensor_tensor(ssq, ssq, tsq, ALU.add)
                # --- rms scale: s = 1/sqrt(mean+eps)  (x 1/sqrt(D) for q) ---
                sq_scale = (D / DIM) if which == "q" else (1.0 / DIM)
                sq_bias = eps_q if which == "q" else eps_k
                inv = small_pool.tile([1, T], F32, tag="inv")
                rt = small_pool.tile([1, T], F32, tag="rt")
                for g in range(2):
                    red = red_ps_pool.tile([1, QG], F32, tag="redps")
                    nc.tensor.matmul(red, ones_col, ssq[:, GS[g]], start=True,
                                     stop=True)
                    nc.scalar.activation(rt[:, GS[g]], red, AF.Sqrt,
                                         bias=sq_bias, scale=sq_scale)
                nc.vector.reciprocal_approx_fast(inv, rt)
                # --- scale + rope -> bf16 dest, per head chunk ---
                for mi in range(H):
                    work = works[mi]
                    dest = dest_tiles[mi]
                    for g in range(2):
                        qs = GS[g]
                        scp = sc_ps_pool.tile([128, QG], F32, tag="scps")
                        nc.tensor.matmul(scp, g_row[:, 128 * mi:128 * (mi + 1)],
                                         inv[:, qs], start=True, stop=True)
                        nc.vector.tensor_tensor(work[:, qs], work[:, qs], scp,
                                                ALU.mult)
                        cc = cs_sb[:, g * QG:(g + 1) * QG]
                        ssg = cs_sb[:, T + g * QG:T + (g + 1) * QG]
                        ta = rope_pool.tile([128, QG], F32, tag="ra")
                        tb = rope_pool.tile([128, QG], F32, tag="rb")
                        sw = rope_pool.tile([128, QG], F32, tag="rsw")
                        nc.vector.tensor_tensor(ta, work[:, qs], cc, ALU.mult)
                        nc.vector.tensor_tensor(tb, work[:, qs], ssg, ALU.mult)
                        nc.sync.dma_start(sw[0:64, :], tb[64:128, :])
                        nc.sync.dma_start(sw[64:128, :], tb[0:64, :])
                        nc.vector.tensor_tensor(dest[:, qs], ta, sw, ALU.add)

            # ---- k first (feeds the first collective) ----
            k_tiles = [att_pool.tile([128, T], BF16, tag=f"att{h}",
                                     name=f"kt{h}") for h in range(H)]
            qk_path("k", k_tiles)
            for mi in range(H):
                nc.sync.dma_start(kcon.ap()[128 * mi:128 * (mi + 1), :],
                                  k_tiles[mi])
            nc.gpsimd.collective_compute(
                "AllGather", ALU.bypass, replica_groups=[list(range(NCORES))],
                ins=[kcon.ap()], outs=[gk.ap()])

            # ---- v: token-major, contraction over dim chunks + bias row ----
            for og in range(HD // 512):
                vb = small_pool.tile([1, 512], BF16, tag="vb")
                nc.sync.dma_start(
                    vb, WvTa.tensor[DIM:DIM + 1, 512 * og:512 * (og + 1)])
                vw = [vw_pool.tile([128, 512], BF16, tag=f"vw{kc}",
                                   name=f"vw{kc}") for kc in range(KQ)]
                for kc in range(KQ):
                    nc.sync.dma_start(
                        vw[kc], WvTa.tensor[128 * kc:128 * (kc + 1),
                                            512 * og:512 * (og + 1)])
                for (t0, tn_) in TOK_SPLITS:
                    ps = ps_pool.tile([128, 1024], F32, tag="qkps")
                    for kc in range(KQ):
                        nc.tensor.matmul(ps[0:tn_, 0:512],
                                         hid[kc][:, t0:t0 + tn_],
                                         vw[kc], start=(kc == 0), stop=False)
                    nc.tensor.matmul(ps[0:tn_, 0:512], hid_ones[:, t0:t0 + tn_],
                                     vb, start=False, stop=True)
                    vsb = tmp_pool.tile([128, 512], BF16, tag="vsb")
                    nc.scalar.activation(vsb[0:tn_, :], ps[0:tn_, 0:512],
                                         AF.Identity)
                    nc.sync.dma_start(
                        vcon.ap()[t0:t0 + tn_, 512 * og:512 * (og + 1)],
                        vsb[0:tn_, :])
            nc.gpsimd.collective_compute(
                "AllGather", ALU.bypass, replica_groups=[list(range(NCORES))],
                ins=[vcon.ap()], outs=[gv.ap()])

            # ---- q ----
            q_tiles = [q_pool.tile([128, T], BF16, tag=f"q{h}", name=f"qt{h}")
                       for h in range(H)]
            qk_path("q", q_tiles)

        # ================= phase B: attention ================================
        with (
            tc.tile_pool(name="kwin", bufs=2) as kv_pool,
            tc.tile_pool(name="vwin", bufs=30) as vt_pool,
            tc.tile_pool(name="probs", bufs=6) as probs_pool,
            tc.tile_pool(name="attm", bufs=2) as attm_pool,
            tc.tile_pool(name="attsc", bufs=2, space="PSUM") as sc_ps,
            tc.tile_pool(name="attop", bufs=1, space="PSUM") as out_ps,
            tc.tile_pool(name="attden", bufs=1, space="PSUM") as den_ps,
        ):
            att_tiles = []
            for h in range(H):
                ksb = kv_pool.tile([128, 4 * T], BF16, tag="ksb")
                for w in range(4):
                    nc.sync.dma_start(
                        ksb[:, w * T:(w + 1) * T],
                        gk[bass.ds(kregs[w] + 128 * h, 128), :])
                vts = []
                for w in range(4):
                    for (t0, tn_) in TOK_SPLITS:
                        vt = vt_pool.tile([128, 128], BF16, tag="vt")
                        nc.sync.dma_start(
                            vt[0:tn_, :],
                            gv[bass.ds(vregs[w] + t0, tn_),
                               128 * h:128 * (h + 1)])
                        vts.append((w, t0, tn_, vt))
                ath = att_pool.tile([128, T], BF16, tag=f"att{h}")
                att_tiles.append(ath)
                op2 = out_ps.tile([128, 1024], F32, tag="op")
                dps = [den_ps.tile([1, QG], F32, tag="dp0", name="dp0"),
                       den_ps.tile([1, QG], F32, tag="dp1", name="dp1")]
                n_kc = len(vts)
                for ci, (w, t0, tn_, vt) in enumerate(vts):
                    sp2 = sc_ps.tile([128, 1024], F32, tag="sp")
                    for g in range(2):
                        nc.tensor.matmul(
                            sp2[0:tn_, PS2[g]],
                            ksb[:, w * T + t0:w * T + t0 + tn_],
                            q_tiles[h][:, GS[g]], start=True, stop=True)
                    pr = probs_pool.tile([128, 2 * QG], BF16, tag="pr")
                    act2(pr[0:tn_, :], sp2[0:tn_, :], AF.Exp)
                    for g in range(2):
                        nc.tensor.matmul(dps[g], ones_bf[0:tn_, :],
                                         pr[0:tn_, GS[g]],
                                         start=(ci == 0), stop=(ci == n_kc - 1))
                    for g in range(2):
                        nc.tensor.matmul(op2[:, PS2[g]], vt[0:tn_, :],
                                         pr[0:tn_, GS[g]],
                                         start=(ci == 0), stop=(ci == n_kc - 1))
                osb = attm_pool.tile([128, 2 * QG], F32, tag="osb")
                act2(osb, op2, AF.Identity)
                dsb = attm_pool.tile([1, 2 * QG], F32, tag="dsb")
                for g in range(2):
                    nc.vector.reciprocal_approx_fast(dsb[:, GS[g]], dps[g])
                bc2 = out_ps.tile([128, 1024], F32, tag="op")
                for g in range(2):
                    nc.tensor.matmul(bc2[:, PS2[g]], ones_row, dsb[:, GS[g]],
                                     start=True, stop=True)
                nc.vector.tensor_tensor(
                    ath.rearrange("p (a b) -> p a b", a=2),
                    osb.rearrange("p (a b) -> p a b", a=2),
                    bc2.rearrange("p (a b) -> p a b", a=2)[:, :, 0:QG],
                    ALU.mult)

        # ================= phase C: output projection ========================
        with (
            tc.tile_pool(name="wos", bufs=3) as wo_pool,
            tc.tile_pool(name="osbp", bufs=2) as o_pool,
            tc.tile_pool(name="opps", bufs=2, space="PSUM") as op_ps,
        ):
            for od in range(DIM // 128):
                wo = wo_pool.tile([128, HD], BF16, tag="wo")
                nc.sync.dma_start(
                    wo.rearrange("p (c m) -> p c m", c=H),
                    WoT.tensor[:, 128 * od:128 * (od + 1)].rearrange(
                        "(c p) m -> p c m", p=128))
                ot = o_pool.tile([128, T], F32, tag="ot")
                ps2 = op_ps.tile([128, 1024], F32, tag="opps")
                for hc in range(H):
                    for g in range(2):
                        nc.tensor.matmul(ps2[:, PS2[g]],
                                         wo[:, 128 * hc:128 * (hc + 1)],
                                         att_tiles[hc][:, GS[g]],
                                         start=(hc == 0), stop=(hc == H - 1))
                act2(ot, ps2, AF.Identity, bias=bo_sb[:, od:od + 1])
                nc.sync.dma_start(outT.tensor[128 * od:128 * (od + 1), :], ot)


_CACHED_NC = None
_LAST_IN_MAPS = None


def _get_nc():
    global _CACHED_NC
    if _CACHED_NC is None:
        _CACHED_NC = _build_nc()
    return _CACHED_NC


def _deinterleave(n):
    """Permutation putting even rotary lanes first within each 128-dim head."""
    idx = np.arange(n).reshape(-1, D)
    return np.concatenate([idx[:, 0::2], idx[:, 1::2]], axis=1).reshape(-1)


def kernel(hidden_states, freqs_cos, freqs_sin, W_qkv, b_qkv, gq, gk, W_out,
           b_out):
    hidden_states = np.asarray(hidden_states, dtype=np.float32)
    freqs_cos = np.asarray(freqs_cos, dtype=np.float32)
    freqs_sin = np.asarray(freqs_sin, dtype=np.float32)
    W_qkv = np.asarray(W_qkv, dtype=np.float32)
    b_qkv = np.asarray(b_qkv, dtype=np.float32)
    gq = np.asarray(gq, dtype=np.float32)
    gk = np.asarray(gk, dtype=np.float32)
    W_out = np.asarray(W_out, dtype=np.float32)
    b_out = np.asarray(b_out, dtype=np.float32)

    nc = _get_nc()

    perm = _deinterleave(HD)
    Wq, Wk, Wv = W_qkv[:HD][perm], W_qkv[HD:2 * HD][perm], W_qkv[2 * HD:]
    bq, bk, bv = b_qkv[:HD][perm], b_qkv[HD:2 * HD][perm], b_qkv[2 * HD:]
    gqp, gkp = gq[perm], gk[perm]

    WqkT = np.ascontiguousarray(
        np.concatenate([Wq, Wk], axis=0).T).astype(NP_BF16)   # [1536, 3072]
    WvTa = np.concatenate([Wv.T, bv[None, :]],
                          axis=0).astype(NP_BF16)             # [1537, 1536]
    bqk = np.ascontiguousarray(
        np.concatenate([bq, bk]).reshape(2 * H, 128).T)       # [128, 24]
    grow = np.concatenate([gqp, gkp])[None, :]                # [1, 3072]
    WoT = np.ascontiguousarray(W_out.T).astype(NP_BF16)       # [1536, 1536]
    bo = np.ascontiguousarray(b_out.reshape(DIM // 128, 128).T)  # [128, 12]

    in_maps = []
    for c in range(NCORES):
        sl = slice(c * T, (c + 1) * T)
        hidT = np.concatenate([
            np.ascontiguousarray(hidden_states[0, sl, :].T),
            np.ones((1, T), np.float32)], axis=0).astype(NP_BF16)  # [1537, 780]
        f = (c * T) // L
        if f == 0:
            win = [0, 1, 0, 1]
        else:
            base = 2 * (f - 1)
            win = [base, base + 1, base + 2, base + 3]
        cc = np.ascontiguousarray(freqs_cos[sl].T)            # [64, 780]
        ss = np.ascontiguousarray(freqs_sin[sl].T)
        csd = np.concatenate([
            np.concatenate([cc, cc], axis=0),
            np.concatenate([ss, -ss], axis=0)], axis=1)       # [128, 1560]
        in_maps.append({
            "hidT": hidT,
            "csd": csd,
            "wink": np.asarray([[w * HD for w in win]], np.uint32),
            "winv": np.asarray([[w * T for w in win]], np.uint32),
            "WqkT": WqkT, "WvTa": WvTa, "bqk": bqk, "grow": grow,
            "WoT": WoT, "bo": bo,
        })

    global _LAST_IN_MAPS
    _LAST_IN_MAPS = in_maps
    res = bass_utils.run_bass_kernel_spmd(nc, in_maps,
                                          core_ids=list(range(NCORES)))
    out = np.empty((1, S, DIM), np.float32)
    for c in range(NCORES):
        out[0, c * T:(c + 1) * T, :] = res.results[c]["outT"].T
    return out
